# revision 16
# baseline (speedup 1.0000x reference)
"""Trainium2 Bass kernel for nn_CrystalRGCNVAE (2-layer basis-RGCN VAE over
256 independent 128-node crystal graphs).

Strategy
--------
Graph-level data parallelism: 32 graphs per NeuronCore x 8 cores, no
collectives (outputs are disjoint).

The message passing (gather + segment_sum over 2048 edges/graph) is
reformulated as dense per-graph matmuls: for relation r and graph g,
  agg_g = A_{g,r} @ (h_g @ W_r),   A_{g,r}[d,s] = #edges s->d of type r.
A is built host-side from (src, dst, edge_types) via one bincount and shipped
as bf16 (small integer counts are exact in bf16). All dense-layer matmuls run
as float32r (TF32) with weight-stationary, 512-column moving operands; the
per-graph aggregation matmuls run bf16-moving to dodge the 4-cycles/row fp32
penalty on short moving operands.

Everything downstream of the graph conv exploits two structural facts:
  * recon_edge rows are identical within a graph (z_exp[src]==z_exp[dst]==
    zp[g]), so it is computed per-graph [B,3] and broadcast.
  * the zero lattice padding lets Wen1/Wst1 drop to their first 32 rows.

Device tensors are kept feature-major ([feat, nodes]) wherever a dense layer
consumes them (weight-stationary streaming); node-major ([node, feat]) where
LayerNorm / aggregation needs them. Host code fixes up output layouts.
"""
import numpy as np
import ml_dtypes
from contextlib import ExitStack

from concourse import bacc, masks
import concourse.bass as bass
import concourse.mybir as mybir
from concourse.tile import TileContext
from concourse.bass_utils import run_bass_kernel_spmd

F32 = mybir.dt.float32
F32R = mybir.dt.float32r
BF16 = mybir.dt.bfloat16
AF = mybir.ActivationFunctionType
ALU = mybir.AluOpType
AX = mybir.AxisListType

B = 256          # graphs in batch
NPG = 128        # nodes per graph
RELS = 4
H1 = 64          # layer-1 width
H2 = 128         # layer-2 width
LAT = 32
NCORES = 8
GPC = B // NCORES          # 32 graphs per core
NPC = GPC * NPG            # 4096 nodes per core
EPG = 2048                 # edges per graph
LN_EPS = 1e-5
BF = ml_dtypes.bfloat16


# ---------------------------------------------------------------- host prep

def _f32(x):
    return np.ascontiguousarray(np.asarray(x), dtype=np.float32)


def _rgcn_wcat(comp, basis, loop, rbias):
    """[in+1, RELS*out + out]: relation blocks | self-loop block, with an
    extra all-ones-driven row carrying rbias into the self-loop block only."""
    W = np.einsum('rb,bio->rio', _f32(comp), _f32(basis))
    cat = np.concatenate([W[r] for r in range(RELS)] + [_f32(loop)], axis=1)
    i, o = cat.shape
    out = np.zeros((i + 1, o), np.float32)
    out[:i] = cat
    out[i, RELS * (o // (RELS + 1)):] = _f32(rbias)
    return out


def _build_adj(src, dst, etyp):
    """AT[s, (g, r, d)] per core, bf16: AT slice [128s,128d] = A_{g,r}^T."""
    src = np.asarray(src).astype(np.int64)
    dst = np.asarray(dst).astype(np.int64)
    etyp = np.asarray(etyp).astype(np.int64)
    g = src // NPG
    s = src % NPG
    d = dst % NPG
    flat = ((g * RELS + etyp) * NPG + d) * NPG + s
    counts = np.bincount(flat, minlength=B * RELS * NPG * NPG)
    counts = counts.reshape(B, RELS, NPG, NPG).astype(BF)   # [g, r, d, s]
    return counts


def _prepare(node_feats, src, dst, edge_types, eps, params):
    p = params
    w1cat = _rgcn_wcat(p['comp1'], p['basis1'], p['loop1'], p['rbias1'])  # [5,320]
    w2cat = _rgcn_wcat(p['comp2'], p['basis2'], p['loop2'], p['rbias2'])  # [65,640]
    counts = _build_adj(src, dst, edge_types)

    nf = _f32(node_feats).reshape(B, NPG, 4)
    eps = _f32(eps)

    ln1_g, ln1_b = _f32(p['ln1_g']), _f32(p['ln1_b'])
    ln2_g, ln2_b = _f32(p['ln2_g']), _f32(p['ln2_b'])
    flags = {
        'ln1_affine': not (np.all(ln1_g == 1.0) and np.all(ln1_b == 0.0)),
        'ln2_affine': not (np.all(ln2_g == 1.0) and np.all(ln2_b == 0.0)),
        'bmu': bool(np.any(_f32(p['bmu']) != 0.0)),
        'be2': bool(np.any(_f32(p['be2']) != 0.0)),
        'gb2': float(np.asarray(p['gb2']).reshape(-1)[0]),
    }

    We1f = _f32(p['We1'])[:H2] + _f32(p['We1'])[H2:]

    def col(v, n):
        return _f32(v).reshape(n, 1)

    shared = {
        'w1cat': w1cat, 'w2cat': w2cat,
        'gW1': _f32(p['gW1']), 'gW2': _f32(p['gW2']),
        'Wnp': _f32(p['Wnp']), 'Wn1': _f32(p['Wn1']), 'Wn2': _f32(p['Wn2']),
        'We1f': We1f, 'We2': _f32(p['We2']),
        'Wmu': _f32(p['Wmu']), 'Wlv': _f32(p['Wlv']), 'Wlp': _f32(p['Wlp']),
        'Wen1': _f32(p['Wen1'])[:LAT], 'Wen2': _f32(p['Wen2']),
        'Wst1': _f32(p['Wst1'])[:LAT], 'Wst2': _f32(p['Wst2']),
        'gb1': col(p['gb1'], H1), 'bnp': col(p['bnp'], H2),
        'bn1': col(p['bn1'], H1), 'bn2': col(p['bn2'], 4),
        'be1': col(p['be1'], H1), 'blv': col(p['blv'], LAT),
        'blp': col(p['blp'], H2), 'ben1': col(p['ben1'], H1),
        'ben2': col(p['ben2'], 2), 'bst1': col(p['bst1'], H1),
        'bst2': col(p['bst2'], 9),
        'gb2col': np.full((NPG, 1), flags['gb2'], np.float32),
    }
    if flags['ln1_affine']:
        shared['ln1grep'] = np.ascontiguousarray(np.broadcast_to(ln1_g, (NPG, H1)))
        shared['ln1brep'] = np.ascontiguousarray(np.broadcast_to(ln1_b, (NPG, H1)))
    if flags['ln2_affine']:
        shared['ln2grep'] = np.ascontiguousarray(np.broadcast_to(ln2_g, (NPG, H2)))
        shared['ln2brep'] = np.ascontiguousarray(np.broadcast_to(ln2_b, (NPG, H2)))
    if flags['bmu']:
        shared['bmu'] = col(p['bmu'], LAT)
    if flags['be2']:
        shared['be2rep'] = np.ascontiguousarray(
            np.broadcast_to(_f32(p['be2']), (GPC, 3)))

    in_maps = []
    for c in range(NCORES):
        g0 = c * GPC
        # AT: counts[g,r,d,s] -> [s, (g r d)]
        at = counts[g0:g0 + GPC].transpose(3, 0, 1, 2).reshape(NPG, GPC * RELS * NPG)
        nft = np.zeros((5, NPC), np.float32)
        nft[:4] = nf[g0:g0 + GPC].transpose(2, 0, 1).reshape(4, NPC)
        nft[4] = 1.0
        m = {'at': np.ascontiguousarray(at),
             'nft': nft,
             'epst': np.ascontiguousarray(eps[g0:g0 + GPC].T)}
        m.update(shared)
        in_maps.append(m)
    return in_maps, flags


# ------------------------------------------------------------- device build

def _build_program(flags):
    nc = bacc.Bacc("TRN2", target_bir_lowering=False, debug=False)

    def din(name, shape, dtype=F32):
        return nc.dram_tensor(name, list(shape), dtype, kind="ExternalInput")

    def dout(name, shape, dtype=F32):
        return nc.dram_tensor(name, list(shape), dtype, kind="ExternalOutput")

    at_d = din('at', [NPG, GPC * RELS * NPG], BF16)
    nft_d = din('nft', [5, NPC], F32R)
    eps_d = din('epst', [LAT, GPC])
    w1_d = din('w1cat', [5, (RELS + 1) * H1], F32R)
    w2_d = din('w2cat', [H1 + 1, (RELS + 1) * H2], F32R)
    wd = {}
    WMATS = [('gW1', (H2, H1)), ('Wnp', (H2, H2)),
             ('Wn1', (H2, H1)), ('We1f', (H2, H1)),
             ('Wn2', (H1, 4)), ('Wmu', (H2, LAT)), ('Wlv', (H2, LAT)),
             ('Wlp', (LAT, H2)), ('Wen1', (LAT, H1)), ('Wen2', (H1, 2)),
             ('Wst1', (LAT, H1)), ('Wst2', (H1, 9))]
    BVECS = [('gW2', (H1, 1)), ('We2', (H1, 3)),
             ('gb1', (H1, 1)), ('bnp', (H2, 1)), ('bn1', (H1, 1)),
             ('bn2', (4, 1)), ('be1', (H1, 1)), ('blv', (LAT, 1)),
             ('blp', (H2, 1)), ('ben1', (H1, 1)), ('ben2', (2, 1)),
             ('bst1', (H1, 1)), ('bst2', (9, 1)), ('gb2col', (NPG, 1))]
    wmat_names = {nm for nm, _ in WMATS}
    for nm, shp in WMATS:
        wd[nm] = din(nm, shp, F32R)
    for nm, shp in BVECS:
        wd[nm] = din(nm, shp)
    if flags['ln1_affine']:
        wd['ln1grep'] = din('ln1grep', (NPG, H1))
        wd['ln1brep'] = din('ln1brep', (NPG, H1))
    if flags['ln2_affine']:
        wd['ln2grep'] = din('ln2grep', (NPG, H2))
        wd['ln2brep'] = din('ln2brep', (NPG, H2))
    if flags['bmu']:
        wd['bmu'] = din('bmu', (LAT, 1))
    if flags['be2']:
        wd['be2rep'] = din('be2rep', (GPC, 3))

    nemb_d = dout('nemb', [NPG, NPC])          # h2, [n, (g f)]
    rnt_d = dout('rnt', [4, NPC])              # recon_node^T [(f),(g n)]
    re_d = dout('re', [GPC, EPG * 3])          # recon_edge per-graph rows
    mu_d = dout('mut', [LAT, GPC])
    lv_d = dout('lvt', [LAT, GPC])
    z_d = dout('zt', [LAT, GPC])
    en_d = dout('ent', [2, GPC])
    st_d = dout('stt', [9, GPC])

    with TileContext(nc) as tc, ExitStack() as ctx:
        wp = ctx.enter_context(tc.tile_pool(name="wp", bufs=1))

        # persistent sbuf
        ident = wp.tile([NPG, NPG], F32, name="ident")
        masks.make_identity(nc, ident[:])
        ones = wp.tile([NPG, 1], F32, name="ones")
        nc.vector.memset(ones[:], 1.0)
        onesrow = wp.tile([1, NPG], F32, name="onesrow")
        nc.vector.memset(onesrow[:], 1.0)
        lneps = wp.tile([NPG, 1], F32, name="lneps")
        nc.vector.memset(lneps[:], LN_EPS)
        w1c = wp.tile([5, (RELS + 1) * H1], F32R, name="w1c")
        nc.sync.dma_start(w1c[:], w1_d[:])
        w2c = wp.tile([H1 + 1, (RELS + 1) * H2], F32R, name="w2c")
        nc.sync.dma_start(w2c[:], w2_d[:])
        wt = {}
        for nm in wd:
            dt_ = F32R if nm in wmat_names else F32
            wt[nm] = wp.tile(list(wd[nm].shape), dt_, name="w_" + nm)
            nc.sync.dma_start(wt[nm][:], wd[nm][:])
        epst = wp.tile([LAT, GPC], F32, name="epst")
        nc.sync.dma_start(epst[:], eps_d[:])

        H2sb = wp.tile([NPG, NPC], F32, name="H2sb")
        H2T = wp.tile([NPG, NPC], F32R, name="H2T")
        GEMB = wp.tile([NPG, GPC], F32R, name="GEMB")
        ZP = wp.tile([H2, GPC], F32R, name="ZP")
        Z = wp.tile([LAT, GPC], F32R, name="Z")

        # ---------------- stage A: both RGCN layers, per graph ----------------
        with ExitStack() as actx:
            ap_ = actx.enter_context(tc.tile_pool(name="apool", bufs=1))
            NFT = ap_.tile([5, NPC], F32R, name="NFT")
            nc.sync.dma_start(NFT[:], nft_d[:])
            ATk = []
            for k in range(4):
                t = ap_.tile([NPG, 4096], BF16, name=f"AT{k}")
                nc.sync.dma_start(t[:], at_d[:, k * 4096:(k + 1) * 4096])
                ATk.append(t)

            pmm = actx.enter_context(tc.tile_pool(name="pmm", bufs=3, space="PSUM"))
            psm = actx.enter_context(tc.tile_pool(name="psm", bufs=2, space="PSUM"))
            sba = actx.enter_context(tc.tile_pool(name="sba", bufs=3))

            def layer_tail(o_ps, loop_ps, width, h_out, g, lnum, affine):
                """o = LeakyLN(agg_ps + loop_ps): write normalized into h_out."""
                lp = sba.tile([NPG, width], F32, tag=f"lp{lnum}",
                              name=f"lp{lnum}_{g}")
                nc.scalar.copy(lp[:], loop_ps)
                o = sba.tile([NPG, width], F32, tag=f"o{lnum}", name=f"o{lnum}_{g}")
                nc.vector.tensor_tensor(o[:], o_ps[:], lp[:], ALU.add)
                hp = sba.tile([NPG, width], F32, tag=f"hp{lnum}", name=f"hp{lnum}_{g}")
                nc.vector.scalar_tensor_tensor(hp[:], o[:], 0.1, o[:],
                                               ALU.mult, ALU.max)
                st6 = sba.tile([NPG, 6], F32, tag="st6", name=f"st6_{lnum}_{g}")
                nc.vector.bn_stats(st6[:], hp[:])
                mv = sba.tile([NPG, 2], F32, tag="mv", name=f"mv_{lnum}_{g}")
                nc.vector.bn_aggr(mv[:], st6[:])
                std = sba.tile([NPG, 1], F32, tag="std", name=f"std_{lnum}_{g}")
                nc.scalar.activation(std[:], mv[:, 1:2], AF.Sqrt,
                                     bias=lneps[0:NPG, :])
                rstd = sba.tile([NPG, 1], F32, tag="rstd", name=f"rstd_{lnum}_{g}")
                nc.vector.reciprocal(rstd[:], std[:])
                nmr = sba.tile([NPG, 1], F32, tag="nmr", name=f"nmr_{lnum}_{g}")
                nc.vector.scalar_tensor_tensor(
                    nmr[:], mv[:, 0:1], -1.0, rstd[:], ALU.mult, ALU.mult)
                if not affine:
                    nc.scalar.activation(h_out, hp[:], AF.Identity,
                                         bias=nmr[:], scale=rstd[:])
                else:
                    hn = sba.tile([NPG, width], F32, tag=f"hn{lnum}",
                                  name=f"hn{lnum}_{g}")
                    nc.scalar.activation(hn[:], hp[:], AF.Identity,
                                         bias=nmr[:], scale=rstd[:])
                    hg = sba.tile([NPG, width], F32, tag=f"hg{lnum}",
                                  name=f"hg{lnum}_{g}")
                    nc.vector.tensor_tensor(hg[:], hn[:], wt[f'ln{lnum}grep'][:],
                                            ALU.mult)
                    nc.vector.tensor_tensor(h_out, hg[:], wt[f'ln{lnum}brep'][:],
                                            ALU.add)

            for g in range(GPC):
                cs, ce = g * NPG, (g + 1) * NPG
                atp = ATk[g // 8]
                acol = (g % 8) * 512
                # ----- layer 1 -----
                x1 = pmm.tile([NPG, 320], F32, tag="mm", name=f"x1_{g}")
                nc.tensor.matmul(x1[:], NFT[:, cs:ce], w1c[:],
                                 start=True, stop=True)
                x1bf = sba.tile([NPG, RELS * H1], BF16, tag="x1bf",
                                name=f"x1bf_{g}")
                nc.scalar.copy(x1bf[:], x1[:, 0:RELS * H1])
                agg1 = psm.tile([NPG, H1], F32, tag="agg", name=f"agg1_{g}")
                for r in range(RELS):
                    nc.tensor.matmul(
                        agg1[:], atp[:, acol + r * NPG: acol + (r + 1) * NPG],
                        x1bf[:, r * H1:(r + 1) * H1],
                        start=(r == 0), stop=(r == RELS - 1))
                h1 = sba.tile([NPG, H1], F32, tag="h1", name=f"h1_{g}")
                layer_tail(agg1, x1[:, RELS * H1:320], H1, h1[:], g, 1,
                           flags['ln1_affine'])
                tr1 = psm.tile([H1, NPG], F32, tag="tr", name=f"tr1_{g}")
                nc.tensor.transpose(tr1[:], h1[:], ident[:])
                h1t = sba.tile([H1 + 1, NPG], F32R, tag="h1t", name=f"h1t_{g}")
                nc.scalar.copy(h1t[0:H1, :], tr1[:])
                nc.scalar.copy(h1t[H1:H1 + 1, :], onesrow[:])
                # ----- layer 2 -----
                x2a = pmm.tile([NPG, 320], F32, tag="mm", name=f"x2a_{g}")
                x2b = pmm.tile([NPG, 320], F32, tag="mm", name=f"x2b_{g}")
                nc.tensor.matmul(x2a[:], h1t[:], w2c[:, 0:320],
                                 start=True, stop=True)
                nc.tensor.matmul(x2b[:], h1t[:], w2c[:, 320:640],
                                 start=True, stop=True)
                x2bf = sba.tile([NPG, RELS * H2], BF16, tag="x2bf",
                                name=f"x2bf_{g}")
                nc.scalar.copy(x2bf[:, 0:320], x2a[:])
                nc.scalar.copy(x2bf[:, 320:512], x2b[:, 0:192])
                agg2 = psm.tile([NPG, H2], F32, tag="agg", name=f"agg2_{g}")
                for r in range(RELS):
                    nc.tensor.matmul(
                        agg2[:], atp[:, acol + r * NPG: acol + (r + 1) * NPG],
                        x2bf[:, r * H2:(r + 1) * H2],
                        start=(r == 0), stop=(r == RELS - 1))
                layer_tail(agg2, x2b[:, 192:320], H2, H2sb[:, cs:ce], g, 2,
                           flags['ln2_affine'])
                tr2 = psm.tile([NPG, NPG], F32, tag="tr", name=f"tr2_{g}")
                nc.tensor.transpose(tr2[:], H2sb[:, cs:ce], ident[:])
                nc.scalar.copy(H2T[:, cs:ce], tr2[:])

        nc.sync.dma_start(nemb_d[:], H2sb[:])

        # ---------------- stage B1: gate + attention pooling + NEP ------------
        NEP = wp.tile([H2, NPC], F32, name="NEP")
        with ExitStack() as b1:
            pg = b1.enter_context(tc.tile_pool(name="pg", bufs=2, space="PSUM"))
            sbg = b1.enter_context(tc.tile_pool(name="sbg", bufs=1))
            # nep chunks (independent of gate path; keeps PE busy)
            for c in range(8):
                nps = pg.tile([H2, 512], F32, tag="nep", name=f"nep_{c}")
                nc.tensor.matmul(nps[:], wt['Wnp'][:],
                                 H2T[:, c * 512:(c + 1) * 512],
                                 start=True, stop=True)
                nc.scalar.activation(NEP[:, c * 512:(c + 1) * 512], nps[:],
                                     AF.Relu, bias=wt['bnp'][:])
            GT1 = sbg.tile([H1, NPC], F32, name="GT1")
            for c in range(8):
                gp = pg.tile([H1, 512], F32, tag="g1", name=f"g1_{c}")
                nc.tensor.matmul(gp[:], wt['gW1'][:],
                                 H2T[:, c * 512:(c + 1) * 512],
                                 start=True, stop=True)
                nc.scalar.activation(GT1[:, c * 512:(c + 1) * 512], gp[:],
                                     AF.Relu, bias=wt['gb1'][:])
            gcol = pg.tile([NPG, GPC], F32, tag="gc", name="gcol", bufs=1)
            for g in range(GPC):
                nc.tensor.matmul(gcol[:, g:g + 1],
                                 GT1[:, g * NPG:(g + 1) * NPG],
                                 wt['gW2'][:], start=True, stop=True)
            gN = sbg.tile([NPG, GPC], F32, name="gN")
            nc.scalar.activation(gN[:], gcol[:], AF.Identity,
                                 bias=wt['gb2col'][:])
            trg = pg.tile([GPC, NPG], F32, tag="trg", name="trg", bufs=1)
            nc.tensor.transpose(trg[:], gN[:], ident[:])
            gates = sbg.tile([GPC, NPG], F32, name="gates")
            nc.scalar.copy(gates[:], trg[:])
            gmax = sbg.tile([GPC, 1], F32, name="gmax")
            nc.vector.tensor_reduce(gmax[:], gates[:], AX.X, ALU.max)
            ngmax = sbg.tile([GPC, 1], F32, name="ngmax")
            nc.vector.tensor_scalar_mul(ngmax[:], gmax[:], -1.0)
            ex = sbg.tile([GPC, NPG], F32, name="ex")
            den = sbg.tile([GPC, 1], F32, name="den")
            nc.scalar.activation(ex[:], gates[:], AF.Exp, bias=ngmax[:],
                                 accum_out=den[:])
            rden = sbg.tile([GPC, 1], F32, name="rden")
            nc.vector.reciprocal(rden[:], den[:])
            att = sbg.tile([GPC, NPG], F32, name="att")
            nc.scalar.activation(att[:], ex[:], AF.Copy, scale=rden[:])
            tra = pg.tile([NPG, GPC], F32, tag="tra", name="tra", bufs=1)
            nc.tensor.transpose(tra[:], att[:], ident[0:GPC, 0:GPC])
            attN = sbg.tile([NPG, GPC], F32, name="attN")
            nc.scalar.copy(attN[:], tra[:])
            gemb_ps = pg.tile([NPG, GPC], F32, tag="ge", name="gemb_ps", bufs=1)
            for g in range(GPC):
                hw = sbg.tile([NPG, NPG], F32, tag="hw", bufs=3, name=f"hw_{g}")
                nc.scalar.activation(hw[:], H2sb[:, g * NPG:(g + 1) * NPG],
                                     AF.Copy, scale=attN[:, g:g + 1])
                nc.tensor.matmul(gemb_ps[:, g:g + 1], hw[:], ones[:],
                                 start=True, stop=True)
            nc.scalar.copy(GEMB[:], gemb_ps[:])

        # ---------------- stage B2: VAE heads -------------------------------
        with ExitStack() as b2:
            ph = b2.enter_context(tc.tile_pool(name="ph", bufs=2, space="PSUM"))
            sbh = b2.enter_context(tc.tile_pool(name="sbh", bufs=1))
            mu_ps = ph.tile([LAT, GPC], F32, tag="h", name="mu_ps")
            nc.tensor.matmul(mu_ps[:], wt['Wmu'][:], GEMB[:],
                             start=True, stop=True)
            MU = sbh.tile([LAT, GPC], F32, name="MU")
            if flags['bmu']:
                mub = sbh.tile([LAT, GPC], F32, name="mub")
                nc.scalar.activation(mub[:], mu_ps[:], AF.Identity,
                                     bias=wt['bmu'][:])
                nc.vector.tensor_scalar(MU[:], mub[:], -5.0, 5.0, ALU.max, ALU.min)
            else:
                nc.vector.tensor_scalar(MU[:], mu_ps[:], -5.0, 5.0, ALU.max, ALU.min)
            nc.sync.dma_start(mu_d[:], MU[:])

            lv_ps = ph.tile([LAT, GPC], F32, tag="h", name="lv_ps")
            nc.tensor.matmul(lv_ps[:], wt['Wlv'][:], GEMB[:],
                             start=True, stop=True)
            lva = sbh.tile([LAT, GPC], F32, name="lva")
            nc.scalar.activation(lva[:], lv_ps[:], AF.Identity, bias=wt['blv'][:])
            LV = sbh.tile([LAT, GPC], F32, name="LV")
            nc.vector.tensor_scalar(LV[:], lva[:], -10.0, 10.0, ALU.max, ALU.min)
            nc.sync.dma_start(lv_d[:], LV[:])

            STD = sbh.tile([LAT, GPC], F32, name="STD")
            nc.scalar.activation(STD[:], LV[:], AF.Exp, scale=0.5)
            est = sbh.tile([LAT, GPC], F32, name="est")
            nc.vector.tensor_tensor(est[:], epst[:], STD[:], ALU.mult)
            nc.vector.tensor_tensor(Z[:], est[:], MU[:], ALU.add)
            nc.sync.dma_start(z_d[:], Z[:].bitcast(F32))

            zp_ps = ph.tile([H2, GPC], F32, tag="zp", name="zp_ps", bufs=1)
            nc.tensor.matmul(zp_ps[:], wt['Wlp'][:], Z[:],
                             start=True, stop=True)
            nc.scalar.activation(ZP[:], zp_ps[:], AF.Relu, bias=wt['blp'][:])

            for nm1, nm2, b1n, b2n, odim, od in (
                    ('Wen1', 'Wen2', 'ben1', 'ben2', 2, en_d),
                    ('Wst1', 'Wst2', 'bst1', 'bst2', 9, st_d)):
                h_ps = ph.tile([H1, GPC], F32, tag="es", name=f"{nm1}_ps")
                nc.tensor.matmul(h_ps[:], wt[nm1][:], Z[:],
                                 start=True, stop=True)
                hsb = sbh.tile([H1, GPC], F32R, name=f"{nm1}_sb")
                nc.scalar.activation(hsb[:], h_ps[:], AF.Relu, bias=wt[b1n][:])
                o_ps = ph.tile([odim, GPC], F32, tag="es2", name=f"{nm2}_ps")
                nc.tensor.matmul(o_ps[:], wt[nm2][:], hsb[:],
                                 start=True, stop=True)
                osb = sbh.tile([odim, GPC], F32, name=f"{nm2}_sb")
                nc.scalar.activation(osb[:], o_ps[:], AF.Identity,
                                     bias=wt[b2n][:])
                nc.sync.dma_start(od[:], osb[:])

            # recon_edge: per-graph row then broadcast across 2048 edges
            re1_ps = ph.tile([H1, GPC], F32, tag="es", name="re1_ps")
            nc.tensor.matmul(re1_ps[:], wt['We1f'][:], ZP[:],
                             start=True, stop=True)
            RE1 = sbh.tile([H1, GPC], F32, name="RE1")
            nc.scalar.activation(RE1[:], re1_ps[:], AF.Relu, bias=wt['be1'][:])
            re2_ps = ph.tile([GPC, 3], F32, tag="es2", name="re2_ps")
            nc.tensor.matmul(re2_ps[:], RE1[:], wt['We2'][:],
                             start=True, stop=True)
            REbc = sbh.tile([GPC, EPG * 3], F32, name="REbc")
            if flags['be2']:
                re2b = sbh.tile([GPC, 3], F32, name="re2b")
                nc.vector.tensor_tensor(re2b[:], re2_ps[:], wt['be2rep'][:],
                                        ALU.add)
                nc.vector.tensor_copy(REbc[:, 0:3], re2b[:])
            else:
                nc.vector.tensor_copy(REbc[:, 0:3], re2_ps[:])
            w = 3
            while w < EPG * 3:
                nc.vector.tensor_copy(REbc[:, w:2 * w], REbc[:, 0:w])
                w *= 2
            nc.sync.dma_start(re_d[:], REbc[:])

        # ---------------- stage B3: node reconstruction ----------------------
        with ExitStack() as b3:
            pn = b3.enter_context(tc.tile_pool(name="pn", bufs=2, space="PSUM"))
            sbn = b3.enter_context(tc.tile_pool(name="sbn", bufs=1))
            SUM = sbn.tile([H2, NPC], F32R, name="SUM")
            for g in range(GPC):
                nc.vector.tensor_scalar_add(SUM[:, g * NPG:(g + 1) * NPG],
                                            NEP[:, g * NPG:(g + 1) * NPG],
                                            ZP[:, g:g + 1].bitcast(F32))
            TT = sbn.tile([H1, NPC], F32R, name="TT")
            for c in range(8):
                t_ps = pn.tile([H1, 512], F32, tag="t", name=f"t_{c}")
                nc.tensor.matmul(t_ps[:], wt['Wn1'][:],
                                 SUM[:, c * 512:(c + 1) * 512],
                                 start=True, stop=True)
                nc.scalar.activation(TT[:, c * 512:(c + 1) * 512], t_ps[:],
                                     AF.Relu, bias=wt['bn1'][:])
            RNT = sbn.tile([4, NPC], F32, name="RNT")
            for c in range(8):
                rn_ps = pn.tile([4, 512], F32, tag="rn", name=f"rn_{c}")
                nc.tensor.matmul(rn_ps[:], wt['Wn2'][:],
                                 TT[:, c * 512:(c + 1) * 512],
                                 start=True, stop=True)
                nc.scalar.activation(RNT[:, c * 512:(c + 1) * 512], rn_ps[:],
                                     AF.Identity, bias=wt['bn2'][:])
            nc.sync.dma_start(rnt_d[:], RNT[:])

    nc.compile()
    return nc


_NC_CACHE = {}


def _get_program(flags):
    key = tuple(sorted(flags.items()))
    if key not in _NC_CACHE:
        _NC_CACHE[key] = _build_program(flags)
    return _NC_CACHE[key]


def _gather(results):
    nemb, rn, re, mu, lv, z, en, st = [], [], [], [], [], [], [], []
    for r in results:
        nemb.append(r['nemb'].reshape(NPG, GPC, H2).transpose(1, 0, 2)
                    .reshape(NPC, H2))
        rn.append(r['rnt'].reshape(4, GPC, NPG).transpose(1, 2, 0)
                  .reshape(NPC, 4))
        re.append(r['re'].reshape(GPC * EPG, 3))
        mu.append(r['mut'].T)
        lv.append(r['lvt'].T)
        z.append(r['zt'].T)
        en.append(r['ent'].T)
        st.append(r['stt'].T)
    cat = lambda x: np.ascontiguousarray(np.concatenate(x, axis=0))
    return {'mu': cat(mu), 'logvar': cat(lv), 'z': cat(z),
            'node_emb': cat(nemb), 'recon_node': cat(rn),
            'recon_edge': cat(re), 'pred_energy': cat(en),
            'pred_stress': cat(st)}


def kernel(node_feats, src, dst, edge_types, eps, params):
    in_maps, flags = _prepare(node_feats, src, dst, edge_types, eps, params)
    nc = _get_program(flags)
    res = run_bass_kernel_spmd(nc, in_maps, list(range(NCORES)))
    return _gather(res.results)
